# revision 13
# baseline (speedup 1.0000x reference)
"""Masked causal self-attention on 8 trn2 NeuronCores.

Problem: x[4,4096,1024] fp32; q/k/v = x @ W{q,k,v}.T (D=64);
out = softmax(causal(q k^T / 8)) v   -> [4, 4096, 64].

Sharding: core = (batch, parity). Each core loads its batch's full x,
builds k/v for all 4096 rows, and computes attention for the 2048 q rows
it owns (alternating 128-row blocks by parity). SPMD requires one
program for all cores, so per-core differences are carried by data only:
  - parity-1 cores receive x with adjacent 128-row blocks swapped, so
    every core's own q-blocks sit at even block positions;
  - the causal masks (which differ under that permutation) are inputs.

On-chip dataflow per core (all matmuls float32r = full PE rate):
  x [rows,E] --PE transpose--> xT [E,rows] --matmul--> kT/vT/qT
  scores are computed transposed: S^T[kv,q] = kT-block.T @ qT
  softmax without max-subtraction (scores ~ N(0,1), exp is safe in fp32),
  masked after exp by multiplying with 0/1 mask tiles; the softmax
  denominators come free from an appended ones-column in the V stationary
  ([v | 1] -> row 64 of the output accumulator is sum(exp)).
  oT accumulates in PSUM over kv blocks, is normalized, transposed back,
  and DMA'd out.
"""

import sys

sys.path.insert(0, "/opt/trn_rl_repo")

import numpy as np

B, S, E, D = 4, 4096, 1024, 64
P = 128
NBLK = S // P            # 32 kv block positions
NITER = 8                # phase-1 iterations, 512 rows each
NSUP = 4                 # phase-2 q superblocks, 512 own q rows each
OWN = S // 2             # own q rows per core

_prog_cache = {}


def _build_program():
    import concourse.mybir as mybir
    from concourse import bacc, tile

    f32r = mybir.dt.float32r
    f32 = mybir.dt.float32
    bf16 = mybir.dt.bfloat16

    nc = bacc.Bacc("TRN2", target_bir_lowering=False, debug=False, num_devices=8)
    x_d = nc.dram_tensor("x", [S, E], f32r, kind="ExternalInput")
    wkv_d = nc.dram_tensor("wkv", [P, 8 * 128], bf16, kind="ExternalInput")
    wq_d = nc.dram_tensor("wq", [P, 8 * 64], bf16, kind="ExternalInput")
    mask_d = nc.dram_tensor("mask", [P, 8 * 128], bf16, kind="ExternalInput")
    ident_d = nc.dram_tensor("ident", [P, P], f32r, kind="ExternalInput")
    identb_d = nc.dram_tensor("identb", [P, P], bf16, kind="ExternalInput")
    ones_d = nc.dram_tensor("ones", [P, NBLK], bf16, kind="ExternalInput")
    y_d = nc.dram_tensor("y", [OWN, D], f32r, kind="ExternalOutput")

    with tile.TileContext(nc) as tc:
        with (
            tc.tile_pool(name="const", bufs=1) as constp,
            tc.tile_pool(name="xin", bufs=3) as xin,
            tc.tile_pool(name="xt", bufs=2) as xtp,
            tc.tile_pool(name="work", bufs=3) as work,
            tc.tile_pool(name="ps_big", bufs=2, space="PSUM") as ps_big,
            tc.tile_pool(name="ps_kv", bufs=1, space="PSUM") as ps_kv,
            tc.tile_pool(name="ps_small", bufs=1, space="PSUM") as ps_small,
            tc.tile_pool(name="ps_qp", bufs=1, space="PSUM") as ps_qp,
            tc.tile_pool(name="ps_o", bufs=1, space="PSUM") as ps_o,
        ):
            # ---- constants / persistent state ----
            ident = constp.tile([P, P], f32r, tag="ident")
            nc.sync.dma_start(ident[:], ident_d.ap())
            identb = constp.tile([P, P], bf16, tag="identb")
            nc.sync.dma_start(identb[:], identb_d.ap())
            wkv_sb = constp.tile([P, 8, 128], bf16, tag="wkv")
            nc.sync.dma_start(wkv_sb[:], wkv_d.ap().rearrange("p (c m) -> p c m", c=8))
            wq_sb = constp.tile([P, 8, 64], bf16, tag="wq")
            nc.sync.dma_start(wq_sb[:], wq_d.ap().rearrange("p (c m) -> p c m", c=8))
            mask_sb = constp.tile([P, 8, 128], bf16, tag="mask")
            nc.sync.dma_start(mask_sb[:], mask_d.ap().rearrange("p (k c) -> p k c", k=8))

            kT_sb = constp.tile([64, S], bf16, tag="kT")
            qT_sb = constp.tile([64, OWN], bf16, tag="qT")
            vOnes = constp.tile([P, NBLK, 65], bf16, tag="vOnes")
            nc.sync.dma_start(vOnes[:, :, 64], ones_d.ap())

            # ---- phase 1 body: load x, transpose, project to kT/vT/qT ----
            def phase1_iter(it):
                r0 = it * 512
                x_nat = xin.tile([P, 4, E], f32r, tag="xnat")
                nc.sync.dma_start(
                    x_nat[:], x_d.ap()[r0 : r0 + 512].rearrange("(i p) e -> p i e", p=P)
                )
                x_bf = xin.tile([P, 4, E], bf16, tag="xbf")
                nc.vector.tensor_copy(x_bf[:], x_nat[:])
                xT = xtp.tile([P, 8, 512], bf16, tag="xT")
                for ec in range(8):
                    pst = ps_big.tile([P, 512], bf16, tag="bigT")
                    for i in range(4):
                        nc.tensor.transpose(
                            pst[:, i * 128 : (i + 1) * 128],
                            x_bf[:, i, ec * 128 : (ec + 1) * 128],
                            identb[:],
                        )
                    if ec < 6:
                        nc.vector.tensor_copy(xT[:, ec, :], pst[:])
                    else:
                        nc.scalar.copy(xT[:, ec, :], pst[:])

                # fused (k|v) projection for all 512 rows
                pkv = ps_kv.tile([P, 512], f32, tag="kv")
                for ec in range(8):
                    nc.tensor.matmul(
                        pkv[:],
                        wkv_sb[:, ec, :],
                        xT[:, ec, :],
                        start=(ec == 0),
                        stop=(ec == 7),
                    )
                nc.vector.tensor_copy(kT_sb[:, r0 : r0 + 512], pkv[0:64, :])
                vt_sb = work.tile([64, 512], bf16, tag="vt")
                nc.vector.tensor_copy(vt_sb[:], pkv[64:128, :])
                pvt = ps_small.tile([P, 256], bf16, tag="small")
                for i in range(4):
                    nc.tensor.transpose(
                        pvt[:, i * 64 : (i + 1) * 64],
                        vt_sb[:, i * 128 : (i + 1) * 128],
                        identb[:64, :64],
                    )
                nc.vector.tensor_copy(
                    vOnes[:, 4 * it : 4 * it + 4, 0:64],
                    pvt[:].rearrange("p (b d) -> p b d", b=4),
                )

                # q projection for the two own (even-position) blocks
                pq = ps_qp.tile([64, 256], f32, tag="qp")
                for ec in range(8):
                    rhs = xT[:, ec, :].rearrange(
                        "p (l two c) -> p two l c", l=2, two=2, c=128
                    )[:, 0]
                    nc.tensor.matmul(
                        pq[:], wq_sb[:, ec, :], rhs, start=(ec == 0), stop=(ec == 7)
                    )
                nc.vector.tensor_copy(qT_sb[:, it * 256 : (it + 1) * 256], pq[:])

            # ---- phase 2 body: attention per 512-own-q-row superblock ----
            def phase2_sup(s):
                nkv = 8 * (s + 1)
                po = ps_o.tile([65, 512], f32, tag="oacc")
                qT_s = qT_sb[:, s * 512 : (s + 1) * 512]
                for kb in range(nkv):
                    k = kb - 8 * s
                    # suffix blocks only reach q column groups t >= k//2
                    c0 = (k // 2) * 128 if k >= 0 else 0
                    pss = ps_big.tile([P, 512], f32, tag="big512")
                    nc.tensor.matmul(
                        pss[:, c0:],
                        kT_sb[:, kb * 128 : (kb + 1) * 128],
                        qT_s[:, c0:],
                        start=True,
                        stop=True,
                    )
                    expT = work.tile([P, 512], bf16, tag="expT")
                    nc.scalar.activation(
                        expT[:, c0:], pss[:, c0:], mybir.ActivationFunctionType.Exp
                    )
                    if k >= 0:
                        # single boundary group: tri (k even) / zeros-or-ones (k odd)
                        nc.vector.tensor_tensor(
                            expT[:, c0 : c0 + 128],
                            expT[:, c0 : c0 + 128],
                            mask_sb[:, k, :],
                            mybir.AluOpType.mult,
                        )
                    nc.tensor.matmul(
                        po[:, c0:],
                        vOnes[:, kb, :],
                        expT[:, c0:],
                        start=(kb == 0),
                        stop=(kb == nkv - 1),
                    )
                # copy [o | sums] to SBUF, transpose back to q-on-partitions
                # (full 128-wide blocks; rows 65:128 are padding), then
                # normalize with per-partition reciprocal scalars
                oraw = work.tile([P, 512], f32r, tag="oraw")
                nc.vector.tensor_copy(oraw[0:65, :], po[:])
                o_sb = work.tile([P, 4, 64], f32r, tag="o")
                for th in range(2):
                    pot = ps_small.tile([P, 2, P], f32r, tag="small")
                    for t2 in range(2):
                        t = 2 * th + t2
                        nc.tensor.transpose(
                            pot[:, t2, :],
                            oraw[:, t * 128 : (t + 1) * 128],
                            ident[:],
                        )
                    rec = work.tile([P, 2, 1], f32, tag="recip")
                    nc.vector.reciprocal(rec[:], pot[:, :, 64:65])
                    for t2 in range(2):
                        nc.vector.tensor_scalar_mul(
                            o_sb[:, 2 * th + t2, :], pot[:, t2, 0:64], rec[:, t2]
                        )
                nc.sync.dma_start(
                    y_d.ap()[s * 512 : (s + 1) * 512].rearrange(
                        "(t tt) d -> tt t d", tt=P
                    ),
                    o_sb[:],
                )

            # interleave: each superblock s needs kv rows from iters 0..2s+1;
            # emitting s right after those iters keeps PE dense (phase-1 work
            # fills phase-2's ACT-bound gaps) and the PE clock warm
            for s in range(NSUP):
                phase1_iter(2 * s)
                phase1_iter(2 * s + 1)
                phase2_sup(s)

    nc.compile()
    return nc


def _host_inputs(x, Wq, Wk, Wv):
    """Build the per-core in_maps (numpy only)."""
    import ml_dtypes

    bf = ml_dtypes.bfloat16
    wkv = np.concatenate([Wk.T, Wv.T], axis=1)  # [E, 128]
    wkv = np.ascontiguousarray(
        wkv.reshape(8, 128, 128).transpose(1, 0, 2).reshape(128, 8 * 128)
    ).astype(bf)
    wq = (Wq.T / np.sqrt(np.float32(D))).astype(np.float32)  # [E, 64], scale folded
    wq = np.ascontiguousarray(
        wq.reshape(8, 128, 64).transpose(1, 0, 2).reshape(128, 8 * 64)
    ).astype(bf)

    tri = np.triu(np.ones((P, P), np.float32))  # keep kv_row tt <= q_row qq
    masks = []
    for p in range(2):
        m = np.zeros((8, P, P), np.float32)
        for k in range(8):
            if k % 2 == 0:
                m[k] = tri
            elif p == 1:
                m[k] = 1.0
        masks.append(
            np.ascontiguousarray(m.transpose(1, 0, 2).reshape(P, 8 * P)).astype(bf)
        )

    swap = np.arange(NBLK).reshape(-1, 2)[:, ::-1].reshape(-1)  # [1,0,3,2,...]
    in_maps = []
    for core in range(8):
        b, p = core // 2, core % 2
        xb = x[b]
        if p == 1:
            xb = xb.reshape(NBLK, P, E)[swap].reshape(S, E)
        in_maps.append(
            {
                "x": np.ascontiguousarray(xb, dtype=np.float32),
                "wkv": wkv,
                "wq": wq,
                "mask": masks[p],
                "ident": np.eye(P, dtype=np.float32),
                "identb": np.eye(P, dtype=np.float32).astype(bf),
                "ones": np.ones((P, NBLK), bf),
            }
        )
    return in_maps


def _assemble(results):
    out = np.empty((B, S, D), np.float32)
    for core in range(8):
        b, p = core // 2, core % 2
        y = np.asarray(results[core]["y"], dtype=np.float32).reshape(16, P, D)
        for j in range(16):
            g = 2 * j + p
            out[b, g * P : (g + 1) * P, :] = y[j]
    return out


def _get_program():
    if "nc" not in _prog_cache:
        _prog_cache["nc"] = _build_program()
    return _prog_cache["nc"]


def run(inputs, trace=False, trace_kwargs=None):
    from concourse import bass_utils

    nc = _get_program()
    in_maps = _host_inputs(
        inputs["x"], inputs["Wq"], inputs["Wk"], inputs["Wv"]
    )
    res = bass_utils.run_bass_kernel_spmd(
        nc,
        in_maps,
        core_ids=list(range(8)),
        trace=trace,
        **(trace_kwargs or {}),
    )
    return _assemble(res.results), res


def kernel(x, Wq, Wk, Wv):
    out, _ = run({"x": x, "Wq": Wq, "Wk": Wk, "Wv": Wv})
    return out


# revision 15
# speedup vs baseline: 1.1470x; 1.1470x over previous
"""Masked causal self-attention on 8 trn2 NeuronCores.

Problem: x[4,4096,1024] fp32; q/k/v = x @ W{q,k,v}.T (D=64);
out = softmax(causal(q k^T / 8)) v   -> [4, 4096, 64].

Sharding: core = (batch, parity). Each core loads its batch's full x,
builds k/v for all 4096 rows, and computes attention for the 2048 q rows
it owns (alternating 128-row blocks by parity). SPMD requires one
program for all cores, so per-core differences are carried by data only:
  - parity-1 cores receive x with adjacent 128-row blocks swapped, so
    every core's own q-blocks sit at even block positions;
  - the causal masks (which differ under that permutation) are inputs.

On-chip dataflow per core (all matmuls float32r = full PE rate):
  x [rows,E] --PE transpose--> xT [E,rows] --matmul--> kT/vT/qT
  scores are computed transposed: S^T[kv,q] = kT-block.T @ qT
  softmax without max-subtraction (scores ~ N(0,1), exp is safe in fp32),
  masked after exp by multiplying with 0/1 mask tiles; the softmax
  denominators come free from an appended ones-column in the V stationary
  ([v | 1] -> row 64 of the output accumulator is sum(exp)).
  oT accumulates in PSUM over kv blocks, is normalized, transposed back,
  and DMA'd out.
"""

import sys

sys.path.insert(0, "/opt/trn_rl_repo")

import numpy as np

B, S, E, D = 4, 4096, 1024, 64
P = 128
NBLK = S // P            # 32 kv block positions
NITER = 8                # phase-1 iterations, 512 rows each
NSUP = 4                 # phase-2 q superblocks, 512 own q rows each
OWN = S // 2             # own q rows per core

_prog_cache = {}


def _build_program():
    import concourse.mybir as mybir
    from concourse import bacc, tile

    f32r = mybir.dt.float32r
    f32 = mybir.dt.float32
    bf16 = mybir.dt.bfloat16

    nc = bacc.Bacc("TRN2", target_bir_lowering=False, debug=False, num_devices=8)
    x_d = nc.dram_tensor("x", [S, E], f32r, kind="ExternalInput")
    wkv_d = nc.dram_tensor("wkv", [P, 8 * 128], bf16, kind="ExternalInput")
    wq_d = nc.dram_tensor("wq", [P, 8 * 64], bf16, kind="ExternalInput")
    mask_d = nc.dram_tensor("mask", [P, 8 * 128], bf16, kind="ExternalInput")
    ident_d = nc.dram_tensor("ident", [P, P], f32r, kind="ExternalInput")
    identb_d = nc.dram_tensor("identb", [P, P], bf16, kind="ExternalInput")
    ones_d = nc.dram_tensor("ones", [P, NBLK], bf16, kind="ExternalInput")
    y_d = nc.dram_tensor("y", [OWN, D], f32r, kind="ExternalOutput")

    with tile.TileContext(nc) as tc:
        with (
            tc.tile_pool(name="const", bufs=1) as constp,
            tc.tile_pool(name="xin", bufs=3) as xin,
            tc.tile_pool(name="xt", bufs=2) as xtp,
            tc.tile_pool(name="work", bufs=3) as work,
            tc.tile_pool(name="ps_big", bufs=2, space="PSUM") as ps_big,
            tc.tile_pool(name="ps_kv", bufs=1, space="PSUM") as ps_kv,
            tc.tile_pool(name="ps_small", bufs=1, space="PSUM") as ps_small,
                        tc.tile_pool(name="ps_o", bufs=2, space="PSUM") as ps_o,
        ):
            # ---- constants / persistent state ----
            ident = constp.tile([P, P], f32r, tag="ident")
            nc.sync.dma_start(ident[:], ident_d.ap())
            identb = constp.tile([P, P], bf16, tag="identb")
            nc.sync.dma_start(identb[:], identb_d.ap())
            wkv_sb = constp.tile([P, 8, 128], bf16, tag="wkv")
            nc.sync.dma_start(wkv_sb[:], wkv_d.ap().rearrange("p (c m) -> p c m", c=8))
            wq_sb = constp.tile([P, 8, 64], bf16, tag="wq")
            nc.sync.dma_start(wq_sb[:], wq_d.ap().rearrange("p (c m) -> p c m", c=8))
            mask_sb = constp.tile([P, 8, 128], bf16, tag="mask")
            nc.sync.dma_start(mask_sb[:], mask_d.ap().rearrange("p (k c) -> p k c", k=8))

            kT_sb = constp.tile([64, S], bf16, tag="kT")
            qT_sb = constp.tile([64, OWN], bf16, tag="qT")
            vOnes = constp.tile([P, NBLK, 65], bf16, tag="vOnes")
            nc.sync.dma_start(vOnes[:, :, 64], ones_d.ap())

            # ---- phase 1 body: load x, transpose, project to kT/vT/qT ----
            def phase1_iter(it):
                r0 = it * 512
                x_nat = xin.tile([P, 4, E], f32r, tag="xnat")
                for i in range(4):
                    nc.sync.dma_start(
                        x_nat[:, i, :],
                        x_d.ap()[r0 + i * P : r0 + (i + 1) * P].rearrange(
                            "(i p) e -> p (i e)", i=1
                        ),
                    )
                x_bf = xin.tile([P, 4, E], bf16, tag="xbf")
                nc.vector.tensor_copy(x_bf[:], x_nat[:])
                xT = xtp.tile([P, 8, 512], bf16, tag="xT")
                for ec in range(8):
                    pst = ps_big.tile([P, 512], bf16, tag="bigT")
                    for i in range(4):
                        nc.tensor.transpose(
                            pst[:, i * 128 : (i + 1) * 128],
                            x_bf[:, i, ec * 128 : (ec + 1) * 128],
                            identb[:],
                        )
                    if ec < 6:
                        nc.vector.tensor_copy(xT[:, ec, :], pst[:])
                    else:
                        nc.scalar.copy(xT[:, ec, :], pst[:])

                # fused (k|v) projection for all 512 rows
                pkv = ps_kv.tile([P, 512], f32, tag="kv")
                for ec in range(8):
                    nc.tensor.matmul(
                        pkv[:],
                        wkv_sb[:, ec, :],
                        xT[:, ec, :],
                        start=(ec == 0),
                        stop=(ec == 7),
                    )
                nc.vector.tensor_copy(kT_sb[:, r0 : r0 + 512], pkv[0:64, :])
                vt_sb = work.tile([64, 512], bf16, tag="vt")
                nc.vector.tensor_copy(vt_sb[:], pkv[64:128, :])
                pvt = ps_small.tile([P, 256], bf16, tag="small")
                for i in range(4):
                    nc.tensor.transpose(
                        pvt[:, i * 64 : (i + 1) * 64],
                        vt_sb[:, i * 128 : (i + 1) * 128],
                        identb[:64, :64],
                    )
                nc.vector.tensor_copy(
                    vOnes[:, 4 * it : 4 * it + 4, 0:64],
                    pvt[:].rearrange("p (b d) -> p b d", b=4),
                )

                # q projection for the two own (even-position) blocks
                pq = ps_kv.tile([64, 256], f32, tag="kv")
                for ec in range(8):
                    rhs = xT[:, ec, :].rearrange(
                        "p (l two c) -> p two l c", l=2, two=2, c=128
                    )[:, 0]
                    nc.tensor.matmul(
                        pq[:], wq_sb[:, ec, :], rhs, start=(ec == 0), stop=(ec == 7)
                    )
                nc.vector.tensor_copy(qT_sb[:, it * 256 : (it + 1) * 256], pq[:])

            # ---- phase 2: segment-based attention ----
            # o_acc[s] accumulates [o | sums] for superblock s in SBUF across
            # kv segments (psum cannot be held open for the whole kernel)
            o_acc = [
                constp.tile([P, 512], f32r, tag=f"oacc{s}", name=f"oacc{s}")
                for s in range(NSUP)
            ]
            seg_first = [True] * NSUP

            def attend_segment(s, kb0, kb1):
                """superblock s attends kv blocks [kb0, kb1)."""
                qT_s = qT_sb[:, s * 512 : (s + 1) * 512]
                po = ps_o.tile([65, 512], f32, tag="po")
                for kb in range(kb0, kb1):
                    k = kb - 8 * s
                    # suffix blocks only reach q column groups t >= k//2
                    c0 = (k // 2) * 128 if k >= 0 else 0
                    pss = ps_big.tile([P, 512], f32, tag="big512")
                    nc.tensor.matmul(
                        pss[:, c0:],
                        kT_sb[:, kb * 128 : (kb + 1) * 128],
                        qT_s[:, c0:],
                        start=True,
                        stop=True,
                    )
                    expT = work.tile([P, 512], bf16, tag="expT")
                    nc.scalar.activation(
                        expT[:, c0:], pss[:, c0:], mybir.ActivationFunctionType.Exp
                    )
                    if k >= 0:
                        # single boundary group: tri (k even) / zeros-or-ones (k odd)
                        nc.vector.tensor_tensor(
                            expT[:, c0 : c0 + 128],
                            expT[:, c0 : c0 + 128],
                            mask_sb[:, k, :],
                            mybir.AluOpType.mult,
                        )
                    nc.tensor.matmul(
                        po[:, c0:],
                        vOnes[:, kb, :],
                        expT[:, c0:],
                        start=(kb == kb0),
                        stop=(kb == kb1 - 1),
                    )
                if seg_first[s]:
                    nc.vector.tensor_copy(o_acc[s][0:65, :], po[:])
                    seg_first[s] = False
                else:
                    nc.vector.tensor_tensor(
                        o_acc[s][0:65, :], o_acc[s][0:65, :], po[:], mybir.AluOpType.add
                    )

            def finish_sup(s):
                # transpose [o | sums] back to q-on-partitions (full 128-wide
                # blocks; rows 65:128 are padding), normalize, store
                o_sb = work.tile([P, 4, 64], f32r, tag="o")
                for th in range(2):
                    pot = ps_small.tile([P, 2, P], f32r, tag="small")
                    for t2 in range(2):
                        t = 2 * th + t2
                        nc.tensor.transpose(
                            pot[:, t2, :],
                            o_acc[s][:, t * 128 : (t + 1) * 128],
                            ident[:],
                        )
                    rec = work.tile([P, 2, 1], f32, tag="recip")
                    nc.vector.reciprocal(rec[:], pot[:, :, 64:65])
                    for t2 in range(2):
                        nc.vector.tensor_scalar_mul(
                            o_sb[:, 2 * th + t2, :], pot[:, t2, 0:64], rec[:, t2]
                        )
                nc.sync.dma_start(
                    y_d.ap()[s * 512 : (s + 1) * 512].rearrange(
                        "(t tt) d -> tt t d", tt=P
                    ),
                    o_sb[:],
                )

            # process x iterations so that late superblocks (long kv spans)
            # get their q early and attend kv segments as they are built;
            # the tail after the last iter shrinks to ~20 kv blocks
            order = [6, 7, 2, 3, 4, 5, 0, 1]
            avail = set()
            attended = [0] * NSUP  # kv blocks [0, attended[s]) ... tracked as sets
            done_kv = [set() for _ in range(NSUP)]
            processed = set()
            for j in order:
                phase1_iter(j)
                processed.add(j)
                avail |= {4 * j + i for i in range(4)}
                for s in range(NSUP):
                    if not (2 * s in processed and 2 * s + 1 in processed):
                        continue
                    span = set(range(8 * (s + 1)))
                    new_kv = sorted((avail & span) - done_kv[s])
                    # contiguous runs
                    run = []
                    for kb in new_kv + [None]:
                        if run and (kb is None or kb != run[-1] + 1):
                            attend_segment(s, run[0], run[-1] + 1)
                            run = []
                        if kb is not None:
                            run.append(kb)
                    done_kv[s] |= set(new_kv)
                    if done_kv[s] == span:
                        finish_sup(s)

    nc.compile()
    return nc


def _host_inputs(x, Wq, Wk, Wv):
    """Build the per-core in_maps (numpy only)."""
    import ml_dtypes

    bf = ml_dtypes.bfloat16
    wkv = np.concatenate([Wk.T, Wv.T], axis=1)  # [E, 128]
    wkv = np.ascontiguousarray(
        wkv.reshape(8, 128, 128).transpose(1, 0, 2).reshape(128, 8 * 128)
    ).astype(bf)
    wq = (Wq.T / np.sqrt(np.float32(D))).astype(np.float32)  # [E, 64], scale folded
    wq = np.ascontiguousarray(
        wq.reshape(8, 128, 64).transpose(1, 0, 2).reshape(128, 8 * 64)
    ).astype(bf)

    tri = np.triu(np.ones((P, P), np.float32))  # keep kv_row tt <= q_row qq
    masks = []
    for p in range(2):
        m = np.zeros((8, P, P), np.float32)
        for k in range(8):
            if k % 2 == 0:
                m[k] = tri
            elif p == 1:
                m[k] = 1.0
        masks.append(
            np.ascontiguousarray(m.transpose(1, 0, 2).reshape(P, 8 * P)).astype(bf)
        )

    swap = np.arange(NBLK).reshape(-1, 2)[:, ::-1].reshape(-1)  # [1,0,3,2,...]
    in_maps = []
    for core in range(8):
        b, p = core // 2, core % 2
        xb = x[b]
        if p == 1:
            xb = xb.reshape(NBLK, P, E)[swap].reshape(S, E)
        in_maps.append(
            {
                "x": np.ascontiguousarray(xb, dtype=np.float32),
                "wkv": wkv,
                "wq": wq,
                "mask": masks[p],
                "ident": np.eye(P, dtype=np.float32),
                "identb": np.eye(P, dtype=np.float32).astype(bf),
                "ones": np.ones((P, NBLK), bf),
            }
        )
    return in_maps


def _assemble(results):
    out = np.empty((B, S, D), np.float32)
    for core in range(8):
        b, p = core // 2, core % 2
        y = np.asarray(results[core]["y"], dtype=np.float32).reshape(16, P, D)
        for j in range(16):
            g = 2 * j + p
            out[b, g * P : (g + 1) * P, :] = y[j]
    return out


def _get_program():
    if "nc" not in _prog_cache:
        _prog_cache["nc"] = _build_program()
    return _prog_cache["nc"]


def run(inputs, trace=False, trace_kwargs=None):
    from concourse import bass_utils

    nc = _get_program()
    in_maps = _host_inputs(
        inputs["x"], inputs["Wq"], inputs["Wk"], inputs["Wv"]
    )
    res = bass_utils.run_bass_kernel_spmd(
        nc,
        in_maps,
        core_ids=list(range(8)),
        trace=trace,
        **(trace_kwargs or {}),
    )
    return _assemble(res.results), res


def kernel(x, Wq, Wk, Wv):
    out, _ = run({"x": x, "Wq": Wq, "Wk": Wk, "Wv": Wv})
    return out
